# revision 12
# baseline (speedup 1.0000x reference)
"""Bahdanau attention + coverage kernel for Trainium2 (8 NeuronCores).

Math (per batch b, L=2048, E=U=512):
    proj[l,u]  = enc[b,l,:] @ We[:,u] + dec[b,:] @ Wh[:,u] + W_b[u]
               + prev_cov[b,l] * Wc[u] + Wc_b[u]
    score[l]   = tanh(proj[l,:]) @ V + V_b
    a          = softmax(score + (1-mask)*(-1e8))      (unnormalized exp; scores
                                                        are bounded by ||V||_1)
    coverage   = a + prev_cov
    context[e] = sum_l a[l] * enc[b,l,e]

Distribution: data-parallel over batch, 8 batches per core.

Device layout (per core):
  - enc is fed TRANSPOSED from host as encT (8, 512, 2048) so that E sits on
    SBUF partitions; the projection runs as PE matmuls
        psum[L=128, U=512] += encT_blk[E=128, L=128].T @ We[E=128, U=512]
    in float32r (full-rate fp32; inputs pre-rounded on host so results are
    exact w.r.t. the rounded operands).
  - the per-batch affine term (base[u] + pc[l]*Wc[u]) is a 5th accumulation
    matmul with K=2: lhsT=[ones; pc_r], rhs=[base_row; wc_row].
    base_row = dec[b] @ Wh + W_b + Wc_b is precomputed on host (tiny).
  - tanh on ScalarE (PSUM -> SBUF), score on VectorE as a fused
    multiply+reduce against a broadcast V tile.
  - softmax: scores [128,16] -> PE transpose -> [16,128]; exp on ScalarE;
    Z via row-sum + ones-matmul broadcast; unnormalized weights are scaled
    by 1/Z at the end (mask bias and V_b are folded into one additive term).
  - context: exp row broadcast to 128 partitions (SBUF->SBUF DMA +
    gpsimd partition_broadcast), then fused multiply+reduce on VectorE
    against the resident encT tiles.
"""

import numpy as np

import concourse.bass as bass
import concourse.bacc as bacc
import concourse.tile as tile
from concourse import mybir
from concourse.bass_utils import run_bass_kernel_spmd

N_CORES = 8
B, L, ENC, HID, UNITS = 64, 2048, 512, 512, 512
BPC = B // N_CORES  # batches per core
NT = L // 128       # l-tiles per batch (16)
F32 = mybir.dt.float32
F32R = mybir.dt.float32r
_ALU = mybir.AluOpType
_ACT = mybir.ActivationFunctionType


def _round_fp32r(x: np.ndarray) -> np.ndarray:
    """Round fp32 to the float32r grid (matches walrus fp32_to_fp32r)."""
    b = np.ascontiguousarray(x, dtype=np.float32).view(np.uint32).astype(np.uint64)
    return ((b + 0x800) & 0xFFFFF000).astype(np.uint32).view(np.float32)


def _build(mode="full"):
    # mode: "full" | "noctx" (skip context math) | "ctx_nopb" (PE-broadcast
    # instead of gpsimd partition_broadcast)
    nc = bacc.Bacc(
        "TRN2", target_bir_lowering=False, debug=False, num_devices=N_CORES
    )
    encT = nc.dram_tensor("enct", [BPC, ENC, L], F32R, kind="ExternalInput").ap()
    we = nc.dram_tensor("we", [ENC, UNITS], F32R, kind="ExternalInput").ap()
    lhst5 = nc.dram_tensor("lhst5", [BPC, 2, L], F32R, kind="ExternalInput").ap()
    rhs5 = nc.dram_tensor("rhs5", [BPC, 128, UNITS], F32R, kind="ExternalInput").ap()
    zpad = nc.dram_tensor("zpad", [128, L], F32R, kind="ExternalInput").ap()
    vbb = nc.dram_tensor("vbb", [128, UNITS], F32, kind="ExternalInput").ap()
    mb = nc.dram_tensor("mb", [BPC, NT, 128], F32, kind="ExternalInput").ap()
    pcn = nc.dram_tensor("pcn", [BPC, NT, 128], F32, kind="ExternalInput").ap()
    id128 = nc.dram_tensor("id128", [128, 128], F32, kind="ExternalInput").ap()
    ones16 = nc.dram_tensor("ones16", [NT, 128], F32, kind="ExternalInput").ap()

    ctx_o = nc.dram_tensor("ctx_o", [BPC, 4, 128], F32, kind="ExternalOutput").ap()
    attn_o = nc.dram_tensor("attn_o", [BPC, NT, 128], F32, kind="ExternalOutput").ap()
    cov_o = nc.dram_tensor("cov_o", [BPC, NT, 128], F32, kind="ExternalOutput").ap()

    with tile.TileContext(nc) as tc:
        with (
            tc.tile_pool(name="const", bufs=1) as constp,
            tc.tile_pool(name="enc", bufs=3) as encp,
            tc.tile_pool(name="smallin", bufs=3) as sip,
            tc.tile_pool(name="tanh", bufs=3) as tanhp,
            tc.tile_pool(name="escr", bufs=1) as escrp,
            tc.tile_pool(name="ebc", bufs=2) as ebcp,
            tc.tile_pool(name="cscr", bufs=1) as cscrp,
            tc.tile_pool(name="smv", bufs=2) as smvp,
            tc.tile_pool(name="ppair", bufs=3, space="PSUM") as ppsum,
            tc.tile_pool(name="psmall", bufs=1, space="PSUM") as spsum,
        ):
            # ---- constants ----
            we_t = []
            for e in range(4):
                t = constp.tile([128, UNITS], F32R, tag=f"we{e}")
                nc.sync.dma_start(t[:], we[e * 128 : (e + 1) * 128, :])
                we_t.append(t)
            vb_t = constp.tile([128, UNITS], F32, tag="vbb")
            nc.sync.dma_start(vb_t[:], vbb[:])
            id_t = constp.tile([128, 128], F32, tag="id128")
            nc.sync.dma_start(id_t[:], id128[:])
            on_t = constp.tile([NT, 128], F32, tag="ones16")
            nc.sync.dma_start(on_t[:], ones16[:])
            lz = []
            for i in range(2):
                t = constp.tile([128, L], F32R, tag=f"lz{i}")
                nc.sync.dma_start(t[:], zpad[:])
                lz.append(t)

            def head(b):
                """Loads + proj matmuls + tanh + score for batch b."""
                encTs = []
                for e in range(4):
                    t = encp.tile([128, L], F32R, tag=f"enc{e}")
                    half = L // 2
                    nc.sync.dma_start(
                        t[:, 0:half], encT[b, e * 128 : (e + 1) * 128, 0:half]
                    )
                    nc.sync.dma_start(
                        t[:, half:L], encT[b, e * 128 : (e + 1) * 128, half:L]
                    )
                    encTs.append(t)
                lh = lz[b % 2]
                nc.sync.dma_start(lh[0:2, :], lhst5[b])
                rh = sip.tile([128, UNITS], F32R, tag="rh")
                nc.sync.dma_start(rh[:], rhs5[b])
                mb16 = sip.tile([NT, 128], F32, tag="mb16")
                nc.sync.dma_start(mb16[:], mb[b])
                pc16 = sip.tile([NT, 128], F32, tag="pc16")
                nc.sync.dma_start(pc16[:], pcn[b])

                s16 = smvp.tile([128, NT], F32, tag="s16")
                for q in range(NT // 2):
                    pp = ppsum.tile([128, 1024], F32, tag="pp")
                    for j in range(2):
                        t = 2 * q + j
                        out = pp[:, j * 512 : (j + 1) * 512]
                        for e in range(4):
                            nc.tensor.matmul(
                                out,
                                encTs[e][:, t * 128 : (t + 1) * 128],
                                we_t[e][:],
                                start=(e == 0),
                                stop=False,
                            )
                        nc.tensor.matmul(
                            out,
                            lh[:, t * 128 : (t + 1) * 128],
                            rh[:],
                            start=False,
                            stop=True,
                        )  # K=128 (zero-padded rows 2..127) for full-rate f32r
                    th = tanhp.tile([128, 1024], F32, tag="th")
                    nc.scalar.activation(th[:], pp[:], _ACT.Tanh)
                    for j in range(2):
                        t = 2 * q + j
                        scr = escrp.tile([128, 512], F32, tag="escr")
                        nc.vector.affine_mul_reduce(
                            out=scr[:],
                            accum_out=s16[:, t : t + 1],
                            in0=th[:, j * 512 : (j + 1) * 512],
                            in1=vb_t[:],
                            scale=1.0,
                            bias=0.0,
                        )
                return {"encTs": encTs, "s16": s16, "mb16": mb16, "pc16": pc16}

            def tail1(b, st):
                """Transpose + softmax scalars + exp(f32r) for batch b."""
                s16, mb16 = st["s16"], st["mb16"]
                sT = spsum.tile([NT, 128], F32, tag="sT")
                nc.tensor.transpose(sT[:], s16[:], id_t[:])
                sm = smvp.tile([NT, 128], F32, tag="sm")
                nc.vector.tensor_add(sm[:], sT[:], mb16[:])
                e16p = smvp.tile([NT, 128], F32, tag="e16")
                nc.scalar.activation(e16p[:], sm[:], _ACT.Exp)
                zp = smvp.tile([NT, 1], F32, tag="zp")
                nc.vector.reduce_sum(zp[:], e16p[:], axis=mybir.AxisListType.X)
                zb = spsum.tile([128, 1], F32, tag="zb")
                nc.tensor.matmul(zb[:], on_t[:], zp[:], start=True, stop=True)
                rz = smvp.tile([128, 1], F32, tag="rz")
                nc.vector.reciprocal(rz[:], zb[:])
                erow = smvp.tile([1, L], F32, tag="erow")
                nc.sync.dma_start(erow[:], e16p[:])
                eB = ebcp.tile([128, L], F32, tag="eB")
                nc.gpsimd.partition_broadcast(eB[:], erow[:])
                st["e16p"] = e16p
                st["rz"] = rz
                st["eB"] = eB

            def tail2(b, st):
                """Attention/coverage outputs + context for batch b."""
                e16p, rz, pc16, encTs = st["e16p"], st["rz"], st["pc16"], st["encTs"]
                a16 = smvp.tile([NT, 128], F32, tag="a16")
                nc.vector.tensor_scalar_mul(a16[:], e16p[:], rz[0:NT, :])
                nc.sync.dma_start(attn_o[b], a16[:])
                cov16 = smvp.tile([NT, 128], F32, tag="cov16")
                nc.vector.tensor_add(cov16[:], a16[:], pc16[:])
                nc.sync.dma_start(cov_o[b], cov16[:])

                # context: fused mul+reduce on DVE against gpsimd-broadcast exp
                eB = st["eB"]
                ctxc = smvp.tile([128, 4], F32, tag="ctxc")
                for e in range(4):
                    cscr = cscrp.tile([128, L], F32, tag="cscr")
                    nc.vector.affine_mul_reduce(
                        out=cscr[:],
                        accum_out=ctxc[:, e : e + 1],
                        in0=encTs[e][:].bitcast(F32),
                        in1=eB[:],
                        scale=1.0,
                        bias=0.0,
                    )
                ctxs = smvp.tile([128, 4], F32, tag="ctxs")
                nc.vector.tensor_scalar_mul(ctxs[:], ctxc[:], rz[:])
                for e in range(4):
                    nc.sync.dma_start(ctx_o[b, e], ctxs[:, e])

            # 3-stage software pipeline: tail2(b-2) | head(b) | tail1(b-1).
            # tail2 first so its PE broadcasts (long-ready) free enc slots
            # before the next batch's matmuls enter the PE queue.
            states = {}
            for b in range(BPC):
                if b >= 2:
                    tail2(b - 2, states[b - 2])
                    del states[b - 2]["encTs"]
                states[b] = head(b)
                if b >= 1:
                    tail1(b - 1, states[b - 1])
            tail2(BPC - 2, states[BPC - 2])
            tail1(BPC - 1, states[BPC - 1])
            tail2(BPC - 1, states[BPC - 1])
    nc.compile()
    return nc


_NC_CACHE = {}


def _get_nc(mode="full"):
    if mode not in _NC_CACHE:
        _NC_CACHE[mode] = _build(mode)
    return _NC_CACHE[mode]


def _prep_inputs(dec_hidden, enc_output, enc_mask, prev_coverage,
                 W_w, W_b, Wc_w, Wc_b, V_w, V_b):
    dec_hidden = np.ascontiguousarray(dec_hidden, np.float32)
    enc_output = np.ascontiguousarray(enc_output, np.float32)
    prev_cov = np.ascontiguousarray(prev_coverage, np.float32)[..., 0]  # (B, L)
    W_w = np.ascontiguousarray(W_w, np.float32)
    Wh, We = W_w[:HID], W_w[HID:]
    wc_row = np.ascontiguousarray(Wc_w, np.float32)[0]     # (U,)
    v_row = np.ascontiguousarray(V_w, np.float32)[:, 0]    # (U,)
    vb = float(np.asarray(V_b, np.float32).reshape(-1)[0])
    bias0 = (np.asarray(W_b, np.float32) + np.asarray(Wc_b, np.float32))  # (U,)

    # host-side tiny precomputation: per-batch base row of the projection
    base = dec_hidden @ Wh + bias0  # (B, U)

    maskf = np.asarray(enc_mask).astype(np.float32)  # (B, L)
    mbias = (1.0 - maskf) * (-1e8) + vb              # (B, L): additive score term

    encT_r = _round_fp32r(enc_output).transpose(0, 2, 1)  # (B, E, L) view
    we_r = _round_fp32r(We)
    pc_r = _round_fp32r(prev_cov)
    base_r = _round_fp32r(base)
    wc_r = _round_fp32r(wc_row)

    lhst5 = np.empty((B, 2, L), np.float32)
    lhst5[:, 0, :] = 1.0
    lhst5[:, 1, :] = pc_r
    rhs5 = np.zeros((B, 128, UNITS), np.float32)
    rhs5[:, 0, :] = base_r
    rhs5[:, 1, :] = wc_r

    vbb = np.broadcast_to(v_row, (128, UNITS))
    id128 = np.eye(128, dtype=np.float32)
    ones16 = np.ones((NT, 128), np.float32)

    in_maps = []
    for c in range(N_CORES):
        s = slice(c * BPC, (c + 1) * BPC)
        in_maps.append({
            "enct": np.ascontiguousarray(encT_r[s]),
            "we": we_r,
            "lhst5": np.ascontiguousarray(lhst5[s]),
            "rhs5": np.ascontiguousarray(rhs5[s]),
            "vbb": np.ascontiguousarray(vbb),
            "mb": np.ascontiguousarray(mbias[s]).reshape(BPC, NT, 128),
            "pcn": np.ascontiguousarray(prev_cov[s]).reshape(BPC, NT, 128),
            "id128": id128,
            "ones16": ones16,
            "zpad": np.zeros((128, L), np.float32),
        })
    return in_maps


def kernel_with_results(trace=False, mode="full", **inputs):
    """Run the kernel; returns ((ctx, attn, cov), BassKernelResults)."""
    nc = _get_nc(mode)
    in_maps = _prep_inputs(**inputs)
    res = run_bass_kernel_spmd(
        nc, in_maps, core_ids=list(range(N_CORES)), trace=trace
    )
    ctx = np.empty((B, ENC), np.float32)
    attn = np.empty((B, L), np.float32)
    cov = np.empty((B, L, 1), np.float32)
    for c in range(N_CORES):
        s = slice(c * BPC, (c + 1) * BPC)
        out = res.results[c]
        ctx[s] = out["ctx_o"].reshape(BPC, ENC)
        attn[s] = out["attn_o"].reshape(BPC, L)
        cov[s] = out["cov_o"].reshape(BPC, L, 1)
    return (ctx, attn, cov), res


def kernel(**inputs):
    outs, _ = kernel_with_results(trace=False, **inputs)
    return outs


# revision 13
# speedup vs baseline: 1.1601x; 1.1601x over previous
"""Bahdanau attention + coverage kernel for Trainium2 (8 NeuronCores).

Math (per batch b, L=2048, E=U=512):
    proj[l,u]  = enc[b,l,:] @ We[:,u] + dec[b,:] @ Wh[:,u] + W_b[u]
               + prev_cov[b,l] * Wc[u] + Wc_b[u]
    score[l]   = tanh(proj[l,:]) @ V + V_b
    a          = softmax(score + (1-mask)*(-1e8))
    coverage   = a + prev_cov
    context[e] = sum_l a[l] * enc[b,l,e]

Distribution: data-parallel over batch, 8 batches per core.

Device design (per core):
  - enc arrives TRANSPOSED from host as encT (8, 512, 2048) so E sits on SBUF
    partitions; the projection runs as float32r PE matmuls (full fp32-rate;
    operands pre-rounded on host so results are exact w.r.t. rounded inputs):
        psum[L=128, U=512] += encT_blk[E=128, L=128].T @ We[E=128, U=512]
  - the per-batch affine term (base[u] + pc[l]*Wc[u]) is a 5th accumulation
    matmul, zero-padded to K=128 (f32r needs K=128 for 1 cyc/row):
    lhsT = [ones; pc_r; 0...], rhs = [base_row; wc_row; 0...].
    base_row = dec[b] @ Wh + W_b + Wc_b is precomputed on host (tiny).
  - tanh on ScalarE (PSUM pair -> SBUF); score on VectorE via the
    AFFINE_MUL_REDUCE custom op against a broadcast V tile.
  - softmax is unnormalized (scores bounded by ||V||); V_b and the mask bias
    fold into one additive [16,128] term. Z comes from a row-sum + ones-matmul
    broadcast; 1/Z is applied at the end.
  - context: exp row tiles are broadcast across partitions by PE indicator
    matmuls into PSUM, then fused multiply+reduce on VectorE against the
    resident encT tiles.
  - 3-stage software pipeline per batch (tail2(b-2) | head(b) | tail1(b-1))
    so no engine's in-order queue stalls on another batch's chain.
"""

import numpy as np

import concourse.bacc as bacc
import concourse.tile as tile
from concourse import mybir
from concourse.bass_utils import run_bass_kernel_spmd

N_CORES = 8
B, L, ENC, HID, UNITS = 64, 2048, 512, 512, 512
BPC = B // N_CORES  # batches per core
NT = L // 128       # l-tiles per batch (16)
F32 = mybir.dt.float32
F32R = mybir.dt.float32r
_ALU = mybir.AluOpType
_ACT = mybir.ActivationFunctionType


def _round_fp32r(x: np.ndarray) -> np.ndarray:
    """Round fp32 to the float32r grid (matches walrus fp32_to_fp32r)."""
    b = np.ascontiguousarray(x, dtype=np.float32).view(np.uint32).astype(np.uint64)
    return ((b + 0x800) & 0xFFFFF000).astype(np.uint32).view(np.float32)


def _build(split_loads=False):
    nc = bacc.Bacc(
        "TRN2", target_bir_lowering=False, debug=False, num_devices=N_CORES
    )
    encT = nc.dram_tensor("enct", [BPC, ENC, L], F32R, kind="ExternalInput").ap()
    we = nc.dram_tensor("we", [ENC, UNITS], F32R, kind="ExternalInput").ap()
    lhst5 = nc.dram_tensor("lhst5", [BPC, 2, L], F32R, kind="ExternalInput").ap()
    rhs5 = nc.dram_tensor("rhs5", [BPC, 128, UNITS], F32R, kind="ExternalInput").ap()
    zpad = nc.dram_tensor("zpad", [128, L], F32R, kind="ExternalInput").ap()
    ind = nc.dram_tensor("ind", [128, L], F32R, kind="ExternalInput").ap()
    vbb = nc.dram_tensor("vbb", [128, UNITS], F32, kind="ExternalInput").ap()
    mb = nc.dram_tensor("mb", [BPC, NT, 128], F32, kind="ExternalInput").ap()
    pcn = nc.dram_tensor("pcn", [BPC, NT, 128], F32, kind="ExternalInput").ap()
    id128 = nc.dram_tensor("id128", [128, 128], F32, kind="ExternalInput").ap()
    ones16 = nc.dram_tensor("ones16", [NT, 128], F32, kind="ExternalInput").ap()

    ctx_o = nc.dram_tensor("ctx_o", [BPC, 4, 128], F32, kind="ExternalOutput").ap()
    attn_o = nc.dram_tensor("attn_o", [BPC, NT, 128], F32, kind="ExternalOutput").ap()
    cov_o = nc.dram_tensor("cov_o", [BPC, NT, 128], F32, kind="ExternalOutput").ap()

    with tile.TileContext(nc) as tc:
        with (
            tc.tile_pool(name="const", bufs=1) as constp,
            tc.tile_pool(name="enc", bufs=4) as encp,
            tc.tile_pool(name="smallin", bufs=3) as sip,
            tc.tile_pool(name="tanh", bufs=3) as tanhp,
            tc.tile_pool(name="escr", bufs=1) as escrp,
            tc.tile_pool(name="cscr", bufs=1) as cscrp,
            tc.tile_pool(name="smv", bufs=2) as smvp,
            tc.tile_pool(name="ppair", bufs=2, space="PSUM") as ppsum,
            tc.tile_pool(name="pebc", bufs=2, space="PSUM") as ebpsum,
            tc.tile_pool(name="psmall", bufs=1, space="PSUM") as spsum,
        ):
            # ---- constants ----
            we_t = []
            for e in range(4):
                t = constp.tile([128, UNITS], F32R, tag=f"we{e}")
                nc.sync.dma_start(t[:], we[e * 128 : (e + 1) * 128, :])
                we_t.append(t)
            vb_t = constp.tile([128, UNITS], F32, tag="vbb")
            nc.sync.dma_start(vb_t[:], vbb[:])
            id_t = constp.tile([128, 128], F32, tag="id128")
            nc.sync.dma_start(id_t[:], id128[:])
            on_t = constp.tile([NT, 128], F32, tag="ones16")
            nc.sync.dma_start(on_t[:], ones16[:])
            lz = []
            for i in range(2):
                t = constp.tile([128, L], F32R, tag=f"lz{i}")
                nc.sync.dma_start(t[:], zpad[:])
                lz.append(t)
            ind_t = constp.tile([128, L], F32R, tag="ind")
            nc.sync.dma_start(ind_t[:], ind[:])
            e16s = []
            for i in range(2):
                t = constp.tile([128, 128], F32R, tag=f"e16z{i}")
                nc.sync.dma_start(t[:], zpad[:, 0:128])
                e16s.append(t)

            def head(b):
                """Loads + proj matmuls + tanh + score for batch b."""
                encTs = []
                for e in range(4):
                    t = encp.tile([128, L], F32R, tag=f"enc{e}")
                    if split_loads:
                        h = L // 2
                        nc.sync.dma_start(
                            t[:, 0:h], encT[b, e * 128 : (e + 1) * 128, 0:h]
                        )
                        nc.sync.dma_start(
                            t[:, h:L], encT[b, e * 128 : (e + 1) * 128, h:L]
                        )
                    else:
                        nc.sync.dma_start(t[:], encT[b, e * 128 : (e + 1) * 128, :])
                    encTs.append(t)
                lh = lz[b % 2]
                nc.sync.dma_start(lh[0:2, :], lhst5[b])
                rh = sip.tile([128, UNITS], F32R, tag="rh")
                nc.sync.dma_start(rh[:], rhs5[b])
                mb16 = sip.tile([NT, 128], F32, tag="mb16")
                nc.sync.dma_start(mb16[:], mb[b])
                pc16 = sip.tile([NT, 128], F32, tag="pc16")
                nc.sync.dma_start(pc16[:], pcn[b])

                s16 = smvp.tile([128, NT], F32, tag="s16")
                for q in range(NT // 2):
                    pp = ppsum.tile([128, 1024], F32, tag="pp")
                    for j in range(2):
                        t = 2 * q + j
                        out = pp[:, j * 512 : (j + 1) * 512]
                        for e in range(4):
                            nc.tensor.matmul(
                                out,
                                encTs[e][:, t * 128 : (t + 1) * 128],
                                we_t[e][:],
                                start=(e == 0),
                                stop=False,
                            )
                        nc.tensor.matmul(
                            out,
                            lh[:, t * 128 : (t + 1) * 128],
                            rh[:],
                            start=False,
                            stop=True,
                        )  # K=128 (zero-padded) keeps f32r at 1 cyc/row
                    th = tanhp.tile([128, 1024], F32, tag="th")
                    nc.scalar.activation(th[:], pp[:], _ACT.Tanh)
                    for j in range(2):
                        t = 2 * q + j
                        scr = escrp.tile([128, 512], F32, tag="escr")
                        nc.vector.affine_mul_reduce(
                            out=scr[:],
                            accum_out=s16[:, t : t + 1],
                            in0=th[:, j * 512 : (j + 1) * 512],
                            in1=vb_t[:],
                            scale=1.0,
                            bias=0.0,
                        )
                return {"encTs": encTs, "s16": s16, "mb16": mb16, "pc16": pc16}

            def tail1(b, st):
                """Transpose + softmax scalars + exp(f32r) for batch b."""
                s16, mb16 = st["s16"], st["mb16"]
                sT = spsum.tile([NT, 128], F32, tag="sT")
                nc.tensor.transpose(sT[:], s16[:], id_t[:])
                sm = smvp.tile([NT, 128], F32, tag="sm")
                nc.vector.tensor_add(sm[:], sT[:], mb16[:])
                e16p = e16s[b % 2]
                nc.scalar.activation(e16p[0:NT, :], sm[:], _ACT.Exp)
                e16f = e16p[0:NT, :].bitcast(F32)
                zp = smvp.tile([NT, 1], F32, tag="zp")
                nc.vector.reduce_sum(zp[:], e16f, axis=mybir.AxisListType.X)
                zb = spsum.tile([128, 1], F32, tag="zb")
                nc.tensor.matmul(zb[:], on_t[:], zp[:], start=True, stop=True)
                rz = smvp.tile([128, 1], F32, tag="rz")
                nc.vector.reciprocal(rz[:], zb[:])
                st["e16p"] = e16p
                st["rz"] = rz

            def tail2(b, st):
                """Attention/coverage outputs + context for batch b."""
                e16p, rz, pc16, encTs = st["e16p"], st["rz"], st["pc16"], st["encTs"]
                a16 = smvp.tile([NT, 128], F32, tag="a16")
                nc.vector.tensor_scalar_mul(
                    a16[:], e16p[0:NT, :].bitcast(F32), rz[0:NT, :]
                )
                nc.sync.dma_start(attn_o[b], a16[:])
                cov16 = smvp.tile([NT, 128], F32, tag="cov16")
                nc.vector.tensor_add(cov16[:], a16[:], pc16[:])
                nc.sync.dma_start(cov_o[b], cov16[:])

                # context: PE-broadcast exp rows, then fused mul+reduce on DVE
                ctxp = smvp.tile([128, NT], F32, tag="ctxp")
                for c in range(4):
                    eBp = ebpsum.tile([128, 512], F32, tag="eBp")
                    for j in range(4):
                        t = 4 * c + j
                        nc.tensor.matmul(
                            eBp[:, j * 128 : (j + 1) * 128],
                            ind_t[:, t * 128 : (t + 1) * 128],
                            e16p[:],
                            start=True,
                            stop=True,
                        )
                    for e in range(4):
                        cscr = cscrp.tile([128, 512], F32, tag="cscr")
                        nc.vector.affine_mul_reduce(
                            out=cscr[:],
                            accum_out=ctxp[:, e * 4 + c : e * 4 + c + 1],
                            in0=encTs[e][:, c * 512 : (c + 1) * 512].bitcast(F32),
                            in1=eBp[:],
                            scale=1.0,
                            bias=0.0,
                        )
                ctxc = smvp.tile([128, 4], F32, tag="ctxc")
                nc.vector.reduce_sum(
                    ctxc[:],
                    ctxp[:].rearrange("p (e c) -> p e c", c=4),
                    axis=mybir.AxisListType.X,
                )
                ctxs = smvp.tile([128, 4], F32, tag="ctxs")
                nc.vector.tensor_scalar_mul(ctxs[:], ctxc[:], rz[:])
                for e in range(4):
                    nc.sync.dma_start(ctx_o[b, e], ctxs[:, e])

            # 3-stage software pipeline: tail2(b-2) | head(b) | tail1(b-1)
            states = {}
            for b in range(BPC):
                if b >= 2:
                    tail2(b - 2, states[b - 2])
                    del states[b - 2]["encTs"]
                states[b] = head(b)
                if b >= 1:
                    tail1(b - 1, states[b - 1])
            tail2(BPC - 2, states[BPC - 2])
            tail1(BPC - 1, states[BPC - 1])
            tail2(BPC - 1, states[BPC - 1])
    nc.compile()
    return nc


_NC_CACHE = {}


def _get_nc(key="full"):
    if key not in _NC_CACHE:
        _NC_CACHE[key] = _build()
    return _NC_CACHE[key]


def _prep_inputs(dec_hidden, enc_output, enc_mask, prev_coverage,
                 W_w, W_b, Wc_w, Wc_b, V_w, V_b):
    dec_hidden = np.ascontiguousarray(dec_hidden, np.float32)
    enc_output = np.ascontiguousarray(enc_output, np.float32)
    prev_cov = np.ascontiguousarray(prev_coverage, np.float32)[..., 0]  # (B, L)
    W_w = np.ascontiguousarray(W_w, np.float32)
    Wh, We = W_w[:HID], W_w[HID:]
    wc_row = np.ascontiguousarray(Wc_w, np.float32)[0]     # (U,)
    v_row = np.ascontiguousarray(V_w, np.float32)[:, 0]    # (U,)
    vb = float(np.asarray(V_b, np.float32).reshape(-1)[0])
    bias0 = (np.asarray(W_b, np.float32) + np.asarray(Wc_b, np.float32))  # (U,)

    # host-side tiny precomputation: per-batch base row of the projection
    base = dec_hidden @ Wh + bias0  # (B, U)

    maskf = np.asarray(enc_mask).astype(np.float32)  # (B, L)
    mbias = (1.0 - maskf) * (-1e8) + vb              # (B, L): additive score term

    encT_r = _round_fp32r(enc_output).transpose(0, 2, 1)  # (B, E, L) view
    we_r = _round_fp32r(We)
    pc_r = _round_fp32r(prev_cov)
    base_r = _round_fp32r(base)
    wc_r = _round_fp32r(wc_row)

    lhst5 = np.empty((B, 2, L), np.float32)
    lhst5[:, 0, :] = 1.0
    lhst5[:, 1, :] = pc_r
    rhs5 = np.zeros((B, 128, UNITS), np.float32)
    rhs5[:, 0, :] = base_r
    rhs5[:, 1, :] = wc_r

    vbb = np.broadcast_to(v_row, (128, UNITS))
    ind_all = np.zeros((128, L), np.float32)
    for t in range(NT):
        ind_all[t, t * 128 : (t + 1) * 128] = 1.0
    id128 = np.eye(128, dtype=np.float32)
    ones16 = np.ones((NT, 128), np.float32)

    in_maps = []
    for c in range(N_CORES):
        s = slice(c * BPC, (c + 1) * BPC)
        in_maps.append({
            "enct": np.ascontiguousarray(encT_r[s]),
            "we": we_r,
            "lhst5": np.ascontiguousarray(lhst5[s]),
            "rhs5": np.ascontiguousarray(rhs5[s]),
            "vbb": np.ascontiguousarray(vbb),
            "mb": np.ascontiguousarray(mbias[s]).reshape(BPC, NT, 128),
            "pcn": np.ascontiguousarray(prev_cov[s]).reshape(BPC, NT, 128),
            "id128": id128,
            "ones16": ones16,
            "zpad": np.zeros((128, L), np.float32),
            "ind": ind_all,
        })
    return in_maps


def kernel_with_results(trace=False, **inputs):
    """Run the kernel; returns ((ctx, attn, cov), BassKernelResults)."""
    nc = _get_nc()
    in_maps = _prep_inputs(**inputs)
    res = run_bass_kernel_spmd(
        nc, in_maps, core_ids=list(range(N_CORES)), trace=trace
    )
    ctx = np.empty((B, ENC), np.float32)
    attn = np.empty((B, L), np.float32)
    cov = np.empty((B, L, 1), np.float32)
    for c in range(N_CORES):
        s = slice(c * BPC, (c + 1) * BPC)
        out = res.results[c]
        ctx[s] = out["ctx_o"].reshape(BPC, ENC)
        attn[s] = out["attn_o"].reshape(BPC, L)
        cov[s] = out["cov_o"].reshape(BPC, L, 1)
    return (ctx, attn, cov), res


def kernel(**inputs):
    outs, _ = kernel_with_results(trace=False, **inputs)
    return outs


# revision 19
# speedup vs baseline: 1.1706x; 1.0090x over previous
"""Bahdanau attention + coverage kernel for Trainium2 (8 NeuronCores).

Math (per batch b, L=2048, E=U=512):
    proj[l,u]  = enc[b,l,:] @ We[:,u] + dec[b,:] @ Wh[:,u] + W_b[u]
               + prev_cov[b,l] * Wc[u] + Wc_b[u]
    score[l]   = tanh(proj[l,:]) @ V + V_b
    a          = softmax(score + (1-mask)*(-1e8))
    coverage   = a + prev_cov
    context[e] = sum_l a[l] * enc[b,l,e]

Distribution: data-parallel over batch, 8 batches per core.

Device design (per core):
  - enc arrives TRANSPOSED from host as encT (8, 512, 2048) so E sits on SBUF
    partitions; the projection runs as float32r PE matmuls (full fp32-rate;
    operands pre-rounded on host so results are exact w.r.t. rounded inputs):
        psum[L=128, U=512] += encT_blk[E=128, L=128].T @ We[E=128, U=512]
  - the per-batch affine term (base[u] + pc[l]*Wc[u]) is a 5th accumulation
    matmul, zero-padded to K=128 (f32r needs K=128 for 1 cyc/row):
    lhsT = [ones; pc_r; 0...], rhs = [base_row; wc_row; 0...].
    base_row = dec[b] @ Wh + W_b + Wc_b is precomputed on host (tiny).
  - tanh on ScalarE (PSUM pair -> SBUF); score on VectorE via the
    AFFINE_MUL_REDUCE custom op against a broadcast V tile.
  - softmax is unnormalized (scores bounded by ||V||); V_b and the mask bias
    fold into one additive [16,128] term. Z comes from a row-sum + ones-matmul
    broadcast; 1/Z is applied at the end.
  - context: exp row tiles are broadcast across partitions by PE indicator
    matmuls into PSUM, then fused multiply+reduce on VectorE against the
    resident encT tiles.
  - 3-stage software pipeline per batch (tail2(b-2) | head(b) | tail1(b-1))
    so no engine's in-order queue stalls on another batch's chain.
"""

import numpy as np

import concourse.bacc as bacc
import concourse.tile as tile
from concourse import mybir
from concourse.bass_utils import run_bass_kernel_spmd

N_CORES = 8
B, L, ENC, HID, UNITS = 64, 2048, 512, 512, 512
BPC = B // N_CORES  # batches per core
NT = L // 128       # l-tiles per batch (16)
F32 = mybir.dt.float32
F32R = mybir.dt.float32r
_ALU = mybir.AluOpType
_ACT = mybir.ActivationFunctionType


def _round_fp32r(x: np.ndarray) -> np.ndarray:
    """Round fp32 to the float32r grid (matches walrus fp32_to_fp32r)."""
    b = np.ascontiguousarray(x, dtype=np.float32).view(np.uint32).astype(np.uint64)
    return ((b + 0x800) & 0xFFFFF000).astype(np.uint32).view(np.float32)


def _build(split_loads=True, gp_bcast=False):
    nc = bacc.Bacc(
        "TRN2", target_bir_lowering=False, debug=False, num_devices=N_CORES
    )
    encT = nc.dram_tensor("enct", [BPC, ENC, L], F32R, kind="ExternalInput").ap()
    we = nc.dram_tensor("we", [ENC, UNITS], F32R, kind="ExternalInput").ap()
    lhst5 = nc.dram_tensor("lhst5", [BPC, 2, L], F32R, kind="ExternalInput").ap()
    rhs5 = nc.dram_tensor("rhs5", [BPC, 128, UNITS], F32R, kind="ExternalInput").ap()
    zpad = nc.dram_tensor("zpad", [128, L], F32R, kind="ExternalInput").ap()
    ind = nc.dram_tensor("ind", [128, L], F32R, kind="ExternalInput").ap()
    vbb = nc.dram_tensor("vbb", [128, UNITS], F32, kind="ExternalInput").ap()
    mb = nc.dram_tensor("mb", [BPC, NT, 128], F32, kind="ExternalInput").ap()
    pcn = nc.dram_tensor("pcn", [BPC, NT, 128], F32, kind="ExternalInput").ap()
    id128 = nc.dram_tensor("id128", [128, 128], F32, kind="ExternalInput").ap()
    ones16 = nc.dram_tensor("ones16", [NT, 128], F32, kind="ExternalInput").ap()

    ctx_o = nc.dram_tensor("ctx_o", [BPC, 4, 128], F32, kind="ExternalOutput").ap()
    attn_o = nc.dram_tensor("attn_o", [BPC, NT, 128], F32, kind="ExternalOutput").ap()
    cov_o = nc.dram_tensor("cov_o", [BPC, NT, 128], F32, kind="ExternalOutput").ap()

    with tile.TileContext(nc) as tc:
        with (
            tc.tile_pool(name="const", bufs=1) as constp,
            tc.tile_pool(name="enc", bufs=4) as encp,
            tc.tile_pool(name="smallin", bufs=3) as sip,
            tc.tile_pool(name="tanh", bufs=3) as tanhp,
            tc.tile_pool(name="escr", bufs=1) as escrp,
            tc.tile_pool(name="cscr", bufs=1) as cscrp,
            tc.tile_pool(name="smv", bufs=2) as smvp,
            tc.tile_pool(name="ppair", bufs=(3 if gp_bcast else 2), space="PSUM") as ppsum,
            tc.tile_pool(name="pebc", bufs=2, space="PSUM") as ebpsum,
            tc.tile_pool(name="ebc", bufs=1) as ebcp,
            tc.tile_pool(name="psmall", bufs=1, space="PSUM") as spsum,
        ):
            # ---- constants ----
            we_t = []
            for e in range(4):
                t = constp.tile([128, UNITS], F32R, tag=f"we{e}")
                nc.sync.dma_start(t[:], we[e * 128 : (e + 1) * 128, :])
                we_t.append(t)
            vb_t = constp.tile([128, UNITS], F32, tag="vbb")
            nc.sync.dma_start(vb_t[:], vbb[:])
            id_t = constp.tile([128, 128], F32, tag="id128")
            nc.sync.dma_start(id_t[:], id128[:])
            on_t = constp.tile([NT, 128], F32, tag="ones16")
            nc.sync.dma_start(on_t[:], ones16[:])
            lz = []
            for i in range(1 if gp_bcast else 2):
                t = constp.tile([128, L], F32R, tag=f"lz{i}")
                nc.sync.dma_start(t[:], zpad[:])
                lz.append(t)
            if not gp_bcast:
                ind_t = constp.tile([128, L], F32R, tag="ind")
                nc.sync.dma_start(ind_t[:], ind[:])
            e16s = []
            for i in range(2):
                t = constp.tile([128, 128], F32R, tag=f"e16z{i}")
                nc.sync.dma_start(t[:], zpad[:, 0:128])
                e16s.append(t)

            def head(b):
                """Loads + proj matmuls + tanh + score for batch b."""
                # small inputs first so the first affine matmul isn't
                # queued behind 4MB of enc loads
                lh = lz[b % len(lz)]
                nc.sync.dma_start(lh[0:2, :], lhst5[b])
                rh = sip.tile([128, UNITS], F32R, tag="rh")
                nc.sync.dma_start(rh[:], rhs5[b])
                mb16 = sip.tile([NT, 128], F32, tag="mb16")
                nc.sync.dma_start(mb16[:], mb[b])
                pc16 = sip.tile([NT, 128], F32, tag="pc16")
                nc.sync.dma_start(pc16[:], pcn[b])
                encTs = []
                for e in range(4):
                    t = encp.tile([128, L], F32R, tag=f"enc{e}")
                    if split_loads:
                        h = L // 2
                        nc.sync.dma_start(
                            t[:, 0:h], encT[b, e * 128 : (e + 1) * 128, 0:h]
                        )
                        nc.sync.dma_start(
                            t[:, h:L], encT[b, e * 128 : (e + 1) * 128, h:L]
                        )
                    else:
                        nc.sync.dma_start(t[:], encT[b, e * 128 : (e + 1) * 128, :])
                    encTs.append(t)

                s16 = smvp.tile([128, NT], F32, tag="s16")
                for q in range(NT // 2):
                    pp = ppsum.tile([128, 1024], F32, tag="pp")
                    for j in range(2):
                        t = 2 * q + j
                        out = pp[:, j * 512 : (j + 1) * 512]
                        for e in range(4):
                            nc.tensor.matmul(
                                out,
                                encTs[e][:, t * 128 : (t + 1) * 128],
                                we_t[e][:],
                                start=(e == 0),
                                stop=False,
                            )
                        nc.tensor.matmul(
                            out,
                            lh[:, t * 128 : (t + 1) * 128],
                            rh[:],
                            start=False,
                            stop=True,
                        )  # K=128 (zero-padded) keeps f32r at 1 cyc/row
                    th = tanhp.tile([128, 1024], F32, tag="th")
                    nc.scalar.activation(th[:], pp[:], _ACT.Tanh)
                    for j in range(2):
                        t = 2 * q + j
                        scr = escrp.tile([128, 512], F32, tag="escr")
                        nc.vector.affine_mul_reduce(
                            out=scr[:],
                            accum_out=s16[:, t : t + 1],
                            in0=th[:, j * 512 : (j + 1) * 512],
                            in1=vb_t[:],
                            scale=1.0,
                            bias=0.0,
                        )
                return {"encTs": encTs, "s16": s16, "mb16": mb16, "pc16": pc16}

            def tail1(b, st):
                """Transpose + softmax scalars + exp(f32r) for batch b."""
                s16, mb16 = st["s16"], st["mb16"]
                sT = spsum.tile([NT, 128], F32, tag="sT")
                nc.tensor.transpose(sT[:], s16[:], id_t[:])
                sm = smvp.tile([NT, 128], F32, tag="sm")
                nc.vector.tensor_add(sm[:], sT[:], mb16[:])
                e16p = e16s[b % 2]
                nc.scalar.activation(e16p[0:NT, :], sm[:], _ACT.Exp)
                e16f = e16p[0:NT, :].bitcast(F32)
                zp = smvp.tile([NT, 1], F32, tag="zp")
                nc.vector.reduce_sum(zp[:], e16f, axis=mybir.AxisListType.X)
                zb = spsum.tile([128, 1], F32, tag="zb")
                nc.tensor.matmul(zb[:], on_t[:], zp[:], start=True, stop=True)
                rz = smvp.tile([128, 1], F32, tag="rz")
                nc.vector.reciprocal(rz[:], zb[:])
                if gp_bcast:
                    erow = smvp.tile([1, L], F32, tag="erow")
                    nc.sync.dma_start(erow[:], e16p[0:NT, :].bitcast(F32))
                    eB = ebcp.tile([128, L], F32, tag="eB")
                    nc.gpsimd.partition_broadcast(eB[:], erow[:])
                    st["eB"] = eB
                st["e16p"] = e16p
                st["rz"] = rz

            def tail2(b, st):
                """Attention/coverage outputs + context for batch b."""
                e16p, rz, pc16, encTs = st["e16p"], st["rz"], st["pc16"], st["encTs"]
                a16 = smvp.tile([NT, 128], F32, tag="a16")
                nc.vector.tensor_scalar_mul(
                    a16[:], e16p[0:NT, :].bitcast(F32), rz[0:NT, :]
                )
                nc.sync.dma_start(attn_o[b], a16[:])
                cov16 = smvp.tile([NT, 128], F32, tag="cov16")
                nc.vector.tensor_add(cov16[:], a16[:], pc16[:])
                nc.sync.dma_start(cov_o[b], cov16[:])

                # context: broadcast exp rows, then fused mul+reduce on DVE
                if gp_bcast:
                    eB = st["eB"]
                    ctxc = smvp.tile([128, 4], F32, tag="ctxc")
                    for e in range(4):
                        cscr2 = cscrp.tile([128, L], F32, tag="cscr2")
                        nc.vector.affine_mul_reduce(
                            out=cscr2[:],
                            accum_out=ctxc[:, e : e + 1],
                            in0=encTs[e][:].bitcast(F32),
                            in1=eB[:],
                            scale=1.0,
                            bias=0.0,
                        )
                else:
                    ctxp = smvp.tile([128, NT], F32, tag="ctxp")
                    for c in range(4):
                        eBp = ebpsum.tile([128, 512], F32, tag="eBp")
                        for j in range(4):
                            t = 4 * c + j
                            nc.tensor.matmul(
                                eBp[:, j * 128 : (j + 1) * 128],
                                ind_t[:, t * 128 : (t + 1) * 128],
                                e16p[:],
                                start=True,
                                stop=True,
                            )
                        for e in range(4):
                            cscr = cscrp.tile([128, 512], F32, tag="cscr")
                            nc.vector.affine_mul_reduce(
                                out=cscr[:],
                                accum_out=ctxp[:, e * 4 + c : e * 4 + c + 1],
                                in0=encTs[e][:, c * 512 : (c + 1) * 512].bitcast(F32),
                                in1=eBp[:],
                                scale=1.0,
                                bias=0.0,
                            )
                    ctxc = smvp.tile([128, 4], F32, tag="ctxc")
                    nc.vector.reduce_sum(
                        ctxc[:],
                        ctxp[:].rearrange("p (e c) -> p e c", c=4),
                        axis=mybir.AxisListType.X,
                    )
                ctxs = smvp.tile([128, 4], F32, tag="ctxs")
                nc.vector.tensor_scalar_mul(ctxs[:], ctxc[:], rz[:])
                for e in range(4):
                    nc.sync.dma_start(ctx_o[b, e], ctxs[:, e])

            def tail2_pair(b0, st0, b1, st1):
                """Interleaved tail2 for the final two batches."""
                outs = {}
                for b, st in ((b0, st0), (b1, st1)):
                    e16p, rz, pc16 = st["e16p"], st["rz"], st["pc16"]
                    a16 = smvp.tile([NT, 128], F32, tag="a16")
                    nc.vector.tensor_scalar_mul(
                        a16[:], e16p[0:NT, :].bitcast(F32), rz[0:NT, :]
                    )
                    nc.sync.dma_start(attn_o[b], a16[:])
                    cov16 = smvp.tile([NT, 128], F32, tag="cov16")
                    nc.vector.tensor_add(cov16[:], a16[:], pc16[:])
                    nc.sync.dma_start(cov_o[b], cov16[:])
                    ctxp_t = smvp.tile([128, NT], F32, tag=f"ctxp{b % 2}")
                    outs[b] = ctxp_t
                for c in range(4):
                    for b, st in ((b0, st0), (b1, st1)):
                        e16p, encTs = st["e16p"], st["encTs"]
                        ctxp = outs[b]
                        eBp = ebpsum.tile([128, 512], F32, tag="eBp")
                        for j in range(4):
                            t = 4 * c + j
                            nc.tensor.matmul(
                                eBp[:, j * 128 : (j + 1) * 128],
                                ind_t[:, t * 128 : (t + 1) * 128],
                                e16p[:],
                                start=True,
                                stop=True,
                            )
                        for e in range(4):
                            cscr = cscrp.tile([128, 512], F32, tag="cscr")
                            nc.vector.affine_mul_reduce(
                                out=cscr[:],
                                accum_out=ctxp[:, e * 4 + c : e * 4 + c + 1],
                                in0=encTs[e][:, c * 512 : (c + 1) * 512].bitcast(F32),
                                in1=eBp[:],
                                scale=1.0,
                                bias=0.0,
                            )
                for b, st in ((b0, st0), (b1, st1)):
                    ctxc = smvp.tile([128, 4], F32, tag="ctxc")
                    nc.vector.reduce_sum(
                        ctxc[:],
                        outs[b][:].rearrange("p (e c) -> p e c", c=4),
                        axis=mybir.AxisListType.X,
                    )
                    ctxs = smvp.tile([128, 4], F32, tag="ctxs")
                    nc.vector.tensor_scalar_mul(ctxs[:], ctxc[:], st["rz"][:])
                    for e in range(4):
                        nc.sync.dma_start(ctx_o[b, e], ctxs[:, e])

            # 3-stage software pipeline: tail2(b-2) | head(b) | tail1(b-1)
            states = {}
            for b in range(BPC):
                if b >= 2:
                    tail2(b - 2, states[b - 2])
                    del states[b - 2]["encTs"]
                states[b] = head(b)
                if b >= 1:
                    tail1(b - 1, states[b - 1])
            # epilogue: finish the last two batches with their context chunks
            # interleaved so PE broadcasts overlap DVE reduces
            tail1(BPC - 1, states[BPC - 1])
            tail2_pair(BPC - 2, states[BPC - 2], BPC - 1, states[BPC - 1])
    nc.compile()
    return nc


_NC_CACHE = {}


def _get_nc(key="full"):
    if key not in _NC_CACHE:
        _NC_CACHE[key] = _build(gp_bcast=(key == "gp"))
    return _NC_CACHE[key]


def _prep_inputs(dec_hidden, enc_output, enc_mask, prev_coverage,
                 W_w, W_b, Wc_w, Wc_b, V_w, V_b):
    dec_hidden = np.ascontiguousarray(dec_hidden, np.float32)
    enc_output = np.ascontiguousarray(enc_output, np.float32)
    prev_cov = np.ascontiguousarray(prev_coverage, np.float32)[..., 0]  # (B, L)
    W_w = np.ascontiguousarray(W_w, np.float32)
    Wh, We = W_w[:HID], W_w[HID:]
    wc_row = np.ascontiguousarray(Wc_w, np.float32)[0]     # (U,)
    v_row = np.ascontiguousarray(V_w, np.float32)[:, 0]    # (U,)
    vb = float(np.asarray(V_b, np.float32).reshape(-1)[0])
    bias0 = (np.asarray(W_b, np.float32) + np.asarray(Wc_b, np.float32))  # (U,)

    # host-side tiny precomputation: per-batch base row of the projection
    base = dec_hidden @ Wh + bias0  # (B, U)

    maskf = np.asarray(enc_mask).astype(np.float32)  # (B, L)
    mbias = (1.0 - maskf) * (-1e8) + vb              # (B, L): additive score term

    encT_r = _round_fp32r(enc_output).transpose(0, 2, 1)  # (B, E, L) view
    we_r = _round_fp32r(We)
    pc_r = _round_fp32r(prev_cov)
    base_r = _round_fp32r(base)
    wc_r = _round_fp32r(wc_row)

    lhst5 = np.empty((B, 2, L), np.float32)
    lhst5[:, 0, :] = 1.0
    lhst5[:, 1, :] = pc_r
    rhs5 = np.zeros((B, 128, UNITS), np.float32)
    rhs5[:, 0, :] = base_r
    rhs5[:, 1, :] = wc_r

    vbb = np.broadcast_to(v_row, (128, UNITS))
    ind_all = np.zeros((128, L), np.float32)
    for t in range(NT):
        ind_all[t, t * 128 : (t + 1) * 128] = 1.0
    id128 = np.eye(128, dtype=np.float32)
    ones16 = np.ones((NT, 128), np.float32)

    in_maps = []
    for c in range(N_CORES):
        s = slice(c * BPC, (c + 1) * BPC)
        in_maps.append({
            "enct": np.ascontiguousarray(encT_r[s]),
            "we": we_r,
            "lhst5": np.ascontiguousarray(lhst5[s]),
            "rhs5": np.ascontiguousarray(rhs5[s]),
            "vbb": np.ascontiguousarray(vbb),
            "mb": np.ascontiguousarray(mbias[s]).reshape(BPC, NT, 128),
            "pcn": np.ascontiguousarray(prev_cov[s]).reshape(BPC, NT, 128),
            "id128": id128,
            "ones16": ones16,
            "zpad": np.zeros((128, L), np.float32),
            "ind": ind_all,
        })
    return in_maps


def kernel_with_results(trace=False, variant="full", **inputs):
    """Run the kernel; returns ((ctx, attn, cov), BassKernelResults)."""
    nc = _get_nc(variant)
    in_maps = _prep_inputs(**inputs)
    res = run_bass_kernel_spmd(
        nc, in_maps, core_ids=list(range(N_CORES)), trace=trace
    )
    ctx = np.empty((B, ENC), np.float32)
    attn = np.empty((B, L), np.float32)
    cov = np.empty((B, L, 1), np.float32)
    for c in range(N_CORES):
        s = slice(c * BPC, (c + 1) * BPC)
        out = res.results[c]
        ctx[s] = out["ctx_o"].reshape(BPC, ENC)
        attn[s] = out["attn_o"].reshape(BPC, L)
        cov[s] = out["cov_o"].reshape(BPC, L, 1)
    return (ctx, attn, cov), res


def kernel(**inputs):
    outs, _ = kernel_with_results(trace=False, **inputs)
    return outs
